# revision 2
# baseline (speedup 1.0000x reference)
"""Trainium2 Bass kernel for the gated equivariant tensor-product layer.

Math (per node z, MUL=64):
  x0 = feats[:, :64], x1[u,i] = feats[:, 64+3u+i], a0 = attrs[:,0], a1 = attrs[:,1:4]
  out0 = ALPHA*( (x0*a0) @ W1 + C*(sum_i x1_i*a1_i) @ W2 )          # [N,128] = s|g
  out1_i = ALPHA*C*( (x0*a1_i) @ W3 + (x1_i*a0) @ W4 )              # [N,64] per i
  out = [ silu(s) | sigmoid(g)[w]*out1_i[w] at col 64+3w+i ]

Design notes:
 - node-major staging (per-node scalars broadcast along free dims), fp16
 - the i-sum of the W2 path is folded into the matmul contraction by
   replicating W2 rows (dt blocks), so no on-chip reduction is needed
 - stacked lhsT [W3; W4] sums both tensor-product paths in PSUM
 - PE transposes to/from feature-major; matmuls in fp16 (fp32 accum)
 - ACT runs Sigmoid only (no act-table thrash); silu = s*sigmoid(s) on DVE

Sharding: pure data parallelism over nodes, 8 cores x 25000 nodes
(padded to 25088 = 49 chunks of 512 per core).
"""

import sys
import numpy as np

sys.path.insert(0, "/opt/trn_rl_repo")

MUL = 64
C3 = 1.0 / np.sqrt(3.0)
ALPHA = 1.0 / np.sqrt(MUL * 1 * 2)

N_CORES = 8
N_PER = 25000
N_PAD = 25600          # 25 * 1024
CHUNK = 1024
N_CHUNKS = N_PAD // CHUNK
G = 8                  # 128-node groups per chunk
P = 128

_BUILT = None


def _build_nc():
    import concourse.bacc as bacc
    import concourse.mybir as mybir
    from concourse.tile import TileContext
    from concourse.masks import make_identity

    f32 = mybir.dt.float32
    f16 = mybir.dt.float16
    MULT = mybir.AluOpType.mult
    AF = mybir.ActivationFunctionType

    nc = bacc.Bacc("TRN2", target_bir_lowering=False, debug=False)

    feats_d = nc.declare_dram_parameter("node_feats", [N_PAD, 256], f32, isOutput=False)
    attrs_d = nc.declare_dram_parameter("node_attrs", [N_PAD, 4], f32, isOutput=False)
    w1_d = nc.declare_dram_parameter("W1", [64, 128], f32, isOutput=False)
    w2_d = nc.declare_dram_parameter("W2", [64, 128], f32, isOutput=False)
    w3_d = nc.declare_dram_parameter("W3", [64, 64], f32, isOutput=False)
    w4_d = nc.declare_dram_parameter("W4", [64, 64], f32, isOutput=False)
    out_d = nc.declare_dram_parameter("out", [N_PAD, 256], f32, isOutput=True)

    with TileContext(nc) as tc:
        wpool = tc.alloc_tile_pool(name="wpool", bufs=1)
        io = tc.alloc_tile_pool(name="io", bufs=4)
        stage = tc.alloc_tile_pool(name="stage", bufs=3)
        rhs = tc.alloc_tile_pool(name="rhs", bufs=3)
        post = tc.alloc_tile_pool(name="post", bufs=3)
        ps_fwd = tc.alloc_tile_pool(name="ps_fwd", bufs=3, space="PSUM")
        ps_mm = tc.alloc_tile_pool(name="ps_mm", bufs=1, space="PSUM")
        ps_bwd = tc.alloc_tile_pool(name="ps_bwd", bufs=2, space="PSUM")

        # --- constants / weights (once) ---
        ident16 = wpool.tile([P, P], f16, tag="ident16")
        make_identity(nc, ident16)
        ident32 = wpool.tile([P, P], f32, tag="ident32")
        make_identity(nc, ident32)

        # fp32 staging for scaling, then cast to fp16 lhsT tiles.
        wtmp = wpool.tile([P, 128], f32, tag="wtmp")
        nc.sync.dma_start(wtmp[0:64, :], w1_d[:, :])
        nc.sync.dma_start(wtmp[64:128, :], w2_d[:, :])
        nc.vector.tensor_scalar_mul(wtmp[0:64, :], wtmp[0:64, :], float(ALPHA))
        nc.vector.tensor_scalar_mul(wtmp[64:128, :], wtmp[64:128, :], float(ALPHA * C3))

        # Wc0 = [alpha*W1 ; alpha*C3*W2], Wc4 = [alpha*C3*W2 ; alpha*C3*W2]
        Wc0 = wpool.tile([P, 128], f16, tag="Wc0")
        Wc4 = wpool.tile([P, 128], f16, tag="Wc4")
        nc.vector.tensor_copy(Wc0[:, :], wtmp[:, :])
        nc.scalar.copy(Wc4[0:64, :], wtmp[64:128, :])
        nc.scalar.copy(Wc4[64:128, :], wtmp[64:128, :])

        wtmp2 = wpool.tile([P, 64], f32, tag="wtmp2")
        nc.sync.dma_start(wtmp2[0:64, :], w3_d[:, :])
        nc.sync.dma_start(wtmp2[64:128, :], w4_d[:, :])
        nc.vector.tensor_scalar_mul(wtmp2[0:64, :], wtmp2[0:64, :], float(ALPHA * C3))
        nc.vector.tensor_scalar_mul(
            wtmp2[64:128, :], wtmp2[64:128, :], float(ALPHA * C3)
        )
        LA = wpool.tile([P, 64], f16, tag="LA")
        nc.vector.tensor_copy(LA[:, :], wtmp2[:, :])

        # attrs for the whole core, loaded once: [p, chunk, g, 4]
        AA = wpool.tile([P, N_CHUNKS, G, 4], f32, tag="AA")
        nc.sync.dma_start(
            AA[:], attrs_d[:, :].rearrange("(c g p) a -> p c g a", p=P, g=G)
        )

        # --- per-chunk pipeline, software-pipelined: staging of chunk ch is
        # emitted before the compute/store phase of chunk ch-1 so each
        # engine's queue interleaves independent work ---
        def stage_phase(ch):
            z0 = ch * CHUNK
            F = io.tile([P, G, 256], f32, tag="feats")
            nc.sync.dma_start(
                F[:], feats_d[z0 : z0 + CHUNK, :].rearrange("(g p) c -> p g c", p=P)
            )
            A = AA[:, ch]  # [128, G, 4]

            # staging tile S (fp16), per-g column layout (640 cols):
            #   [ t0 | dt_0 dt_1 dt_2 | t3_0 t4_0 | t3_1 t4_1 | t3_2 t4_2 ]
            S = stage.tile([P, G, 640], f16, tag="S")

            # per-i staging ops: broadcasts only on the innermost free dim
            # (middle-dim step-0 APs measured 3-5x slower on DVE/GPSIMD)
            x1v = F[:, :, 64:256].rearrange("p g (u i) -> p g i u", i=3)
            a0b = A[:, :, 0:1].to_broadcast([P, G, 64])
            # t0 = x0 * a0                                   (GPSIMD, contiguous)
            nc.gpsimd.tensor_tensor(S[:, :, 0:64], F[:, :, 0:64], a0b, MULT)
            for i in range(3):
                a1b = A[:, :, 1 + i : 2 + i].to_broadcast([P, G, 64])
                # t3_i = x0 * a1_i                           (GPSIMD, contiguous)
                nc.gpsimd.tensor_tensor(
                    S[:, :, 256 + 128 * i : 320 + 128 * i], F[:, :, 0:64], a1b, MULT
                )
                # dt_i = x1_i * a1_i                         (DVE, strided)
                nc.vector.tensor_tensor(
                    S[:, :, 64 + 64 * i : 128 + 64 * i], x1v[:, :, i, :], a1b, MULT
                )
                # t4_i = x1_i * a0   (i=0,1 DVE; i=2 GPSIMD)
                eng = nc.vector if i < 2 else nc.gpsimd
                eng.tensor_tensor(
                    S[:, :, 320 + 128 * i : 384 + 128 * i], x1v[:, :, i, :], a0b, MULT
                )

            return S

        def compute_phase(ch, S):
            z0 = ch * CHUNK
            # forward transposes (PE, fp16)
            Rt = rhs.tile([P, 5, CHUNK], f16, tag="R")
            for b in range(5):
                FT = ps_fwd.tile([P, CHUNK], f16, tag="ft")
                for g in range(G):
                    nc.tensor.transpose(
                        FT[:, g * P : (g + 1) * P],
                        S[:, g, b * 128 : (b + 1) * 128],
                        ident16,
                    )
                if b in (0, 2):
                    nc.vector.tensor_copy(Rt[:, b, 0:512], FT[:, 0:512])
                    nc.vector.tensor_copy(Rt[:, b, 512:1024], FT[:, 512:1024])
                else:
                    nc.scalar.copy(Rt[:, b, 0:512], FT[:, 0:512])
                    nc.scalar.copy(Rt[:, b, 512:1024], FT[:, 512:1024])

            # matmuls (fp16 in, fp32 accum), per-512 halves for psum rotation
            # blocks: R0=[t0|dt_0] R1=[dt_1|dt_2] R2=[t3_0|t4_0] R3=[t3_1|t4_1] R4=[t3_2|t4_2]
            U = post.tile([P, CHUNK], f32, tag="U")      # sigmoid(g), both halves
            SGS = post.tile([64, CHUNK], f32, tag="SGS")  # sigmoid(s)
            BA = post.tile([P, CHUNK], f16, tag="BA")    # [gated_1 ; gated_0]
            BB = post.tile([P, CHUNK], f16, tag="BB")    # [silu ; gated_2]
            for h in range(CHUNK // 512):
                hs = slice(h * 512, (h + 1) * 512)
                O1 = ps_mm.tile([P, 512], f32, tag="O1")   # [s ; g]
                P1 = ps_mm.tile([P, 512], f32, tag="P1")   # [out1_1 ; out1_0]
                P2 = ps_mm.tile([P, 512], f32, tag="P2")   # [g-dup ; out1_2]
                nc.tensor.matmul(
                    O1[:, :], Wc0[:, :], Rt[:, 0, hs], start=True, stop=False
                )
                nc.tensor.matmul(
                    O1[:, :], Wc4[:, :], Rt[:, 1, hs], start=False, stop=True
                )
                nc.tensor.matmul(
                    P2[0:64, :], Wc0[:, 64:128], Rt[:, 0, hs], start=True, stop=False
                )
                nc.tensor.matmul(
                    P2[0:64, :], Wc4[:, 64:128], Rt[:, 1, hs], start=False, stop=True
                )
                nc.tensor.matmul(P1[0:64, :], LA[:, :], Rt[:, 3, hs])
                nc.tensor.matmul(P1[64:128, :], LA[:, :], Rt[:, 2, hs])
                nc.tensor.matmul(P2[64:128, :], LA[:, :], Rt[:, 4, hs])

                # sigmoids (ACT only ever runs Sigmoid -> no act-table reloads)
                nc.scalar.activation(U[0:64, hs], P2[0:64, :], AF.Sigmoid)
                nc.scalar.activation(U[64:128, hs], O1[64:128, :], AF.Sigmoid)
                nc.scalar.activation(SGS[:, hs], O1[0:64, :], AF.Sigmoid)

                # gating (DVE, psum x sbuf), fp16 out
                nc.vector.tensor_tensor(BA[:, hs], P1[:, :], U[:, hs], MULT)
                nc.vector.tensor_tensor(BB[0:64, hs], O1[0:64, :], SGS[:, hs], MULT)
                nc.vector.tensor_tensor(
                    BB[64:128, hs], P2[64:128, :], U[64:128, hs], MULT
                )

            # backward transposes (PE, fp16) -> node-major psum
            BTA = ps_bwd.tile([P, CHUNK], f16, tag="bt")
            BTB = ps_bwd.tile([P, CHUNK], f16, tag="bt")
            for g in range(G):
                nc.tensor.transpose(
                    BTA[:, g * P : (g + 1) * P], BA[:, g * P : (g + 1) * P], ident16
                )
            for g in range(G):
                nc.tensor.transpose(
                    BTB[:, g * P : (g + 1) * P], BB[:, g * P : (g + 1) * P], ident16
                )

            # final node-major assembly (interleave gated cols 64+3w+i)
            OB = io.tile([P, G, 256], f32, tag="outbuf")
            bav = BTA[:].rearrange("p (g c) -> p g c", g=G)
            bbv = BTB[:].rearrange("p (g c) -> p g c", g=G)
            # BTA cols: [gated_1 | gated_0] -> out cols 64+3w+1 / 64+3w+0
            gpair = OB[:, :, 64:256].rearrange("p g (w i) -> p g i w", i=3)
            nc.vector.tensor_copy(gpair[:, :, 1, :], bav[:, :, 0:64])
            nc.vector.tensor_copy(gpair[:, :, 0, :], bav[:, :, 64:128])
            nc.scalar.copy(OB[:, :, 0:64], bbv[:, :, 0:64])
            nc.scalar.copy(gpair[:, :, 2, :], bbv[:, :, 64:128])

            nc.scalar.dma_start(
                out_d[z0 : z0 + CHUNK, :].rearrange("(g p) c -> p g c", p=P),
                OB[:],
            )

        pend = {}
        for ch in range(N_CHUNKS + 1):
            if ch < N_CHUNKS:
                pend[ch] = stage_phase(ch)
            if ch - 1 in pend:
                compute_phase(ch - 1, pend.pop(ch - 1))

        for pool in (ps_bwd, ps_mm, ps_fwd, post, rhs, stage, io, wpool):
            pool.release()

    nc.compile()
    return nc


def _get_nc():
    global _BUILT
    if _BUILT is None:
        _BUILT = _build_nc()
    return _BUILT


def kernel(node_feats, node_attrs, W1, W2, W3, W4):
    from concourse.bass_utils import run_bass_kernel_spmd

    nc = _get_nc()

    node_feats = np.ascontiguousarray(node_feats, dtype=np.float32)
    node_attrs = np.ascontiguousarray(node_attrs, dtype=np.float32)
    in_maps = []
    for c in range(N_CORES):
        f = node_feats[c * N_PER : (c + 1) * N_PER]
        a = node_attrs[c * N_PER : (c + 1) * N_PER]
        fpad = np.zeros((N_PAD, 256), np.float32)
        apad = np.zeros((N_PAD, 4), np.float32)
        fpad[:N_PER] = f
        apad[:N_PER] = a
        in_maps.append(
            {
                "node_feats": fpad,
                "node_attrs": apad,
                "W1": np.ascontiguousarray(W1, np.float32),
                "W2": np.ascontiguousarray(W2, np.float32),
                "W3": np.ascontiguousarray(W3, np.float32),
                "W4": np.ascontiguousarray(W4, np.float32),
            }
        )

    res = run_bass_kernel_spmd(nc, in_maps, list(range(N_CORES)))
    global LAST_RESULT
    LAST_RESULT = res
    outs = [r["out"][:N_PER] for r in res.results]
    return np.concatenate(outs, axis=0)


LAST_RESULT = None

